# revision 5
# baseline (speedup 1.0000x reference)
"""Trainium2 Bass kernel for nn_CausalFullAttention_37821482009327.

Causal full attention (no softmax) with data-dependent complex relative
position decay, silu gating, and output projection.

Sharding: tensor-parallel over the 16 heads -> 2 heads per NeuronCore x 8.
Each core computes its heads' attention and a partial out-projection; the
host sums the 8 partials and adds b_out (the "all-reduce" at gather time).

v2 design vs baseline:
- a-chain in polar form: phase via Arctan + segmented cumsum (HW
  tensor_tensor_scan), magnitude via sigmoid + segmented cumprod
  (max/mult scan), acr = R * cos(Phi) with cody-waite range reduction.
  Replaces the 38-op complex doubling scan with ~16 ops.
- q/k/v/g/out matmuls in bf16 (x stays exact fp32 through the transpose
  for the precision-critical a-projection, which stays fp32 LOW_HIGH).
- qk pairs row-packed via base_partition 0/64 (concurrent in the PE).
- AV uses zero-padded stationaries so both heads accumulate into one
  full-partition PSUM bank.
- PE warmup matmuls at t=0 against the HAM clock gate.

Shapes (hardcoded): B=1, N=2048, D=1024, H=16, Dh=64, Dc=32.
"""
import sys

sys.path.insert(0, "/opt/trn_rl_repo")

import math

import numpy as np

import concourse.bass as bass
import concourse.tile as tile
from concourse import bacc, mybir
from concourse.bass_utils import run_bass_kernel_spmd
from concourse.masks import make_identity

F32 = mybir.dt.float32
BF16 = mybir.dt.bfloat16
AF = mybir.ActivationFunctionType
ALU = mybir.AluOpType

N = 2048
D = 1024
H_LOC = 2          # heads per core
DH = 64
DC = 32
NCORES = 8
EPS = 1e-10

NCH = N // 128     # 16 n-chunks of 128
DCH = D // 128     # 8 d-chunks of 128
NC4 = N // 512     # 4 n-chunks of 512

MAGIC = float(1.5 * 2 ** 23)
# 2*pi = C1 + C2 + C3 cody-waite split
C1 = float(np.float32(6.28125))
C2 = float(np.float32(0.0019302368))
C3 = float(2 * np.pi - 6.28125 - float(np.float32(0.0019302368)))

N_WARM = 28        # PE warmup matmuls (HAM clock-gate)


def _emit(nc):
    X = nc.dram_tensor("X", [N, D], F32, kind="ExternalInput")
    WQ = nc.dram_tensor("WQ", [D, 128], BF16, kind="ExternalInput")
    WK = nc.dram_tensor("WK", [D, 128], BF16, kind="ExternalInput")
    WA = nc.dram_tensor("WA", [D, 128], F32, kind="ExternalInput")
    WV = nc.dram_tensor("WV", [D, 128], BF16, kind="ExternalInput")
    WG = nc.dram_tensor("WG", [D, 128], BF16, kind="ExternalInput")
    WO = nc.dram_tensor("WO", [128, D], BF16, kind="ExternalInput")
    OUT = nc.dram_tensor("OUT", [D, N], F32, kind="ExternalOutput")

    with (
        tile.TileContext(nc) as tc,
        tc.tile_pool(name="pers", bufs=1) as pers,
        tc.tile_pool(name="ps", bufs=2, space="PSUM") as ps,
        tc.tile_pool(name="xnat", bufs=4) as xnat,
        tc.tile_pool(name="aep", bufs=3) as aep,
        tc.tile_pool(name="ach", bufs=2) as ach,
        tc.tile_pool(name="sse", bufs=6) as sse,
        tc.tile_pool(name="chk", bufs=2) as chk,
        tc.tile_pool(name="gte", bufs=2) as gte,
    ):
        # ---- persistent SBUF tensors ----
        ident = pers.tile([128, 128], F32, tag="ident")
        make_identity(nc, ident[:])
        identb = pers.tile([128, 128], BF16, tag="identb")
        make_identity(nc, identb[:])

        # causal masks for the 4 diagonal offsets: keep j <= i
        maskt = pers.tile([128, 4 * 512], F32, tag="maskt")
        for off in range(4):
            m = maskt[:, off * 512:(off + 1) * 512]
            nc.gpsimd.memset(m, 1.0)
            nc.gpsimd.affine_select(
                out=m, in_=m, compare_op=ALU.is_ge,
                fill=0.0, base=-128 * off, pattern=[[1, 512]],
                channel_multiplier=-1)

        # scan segment masks (free layout (s4, h2, d32); segment = d)
        mask0 = pers.tile([128, 256], F32, tag="mask0")   # 0 at d==0 else 1
        nc.gpsimd.memset(mask0[:], 1.0)
        nc.gpsimd.memset(
            mask0[:].rearrange("p (g d) -> p g d", d=32)[:, :, 0:1], 0.0)
        maskP = pers.tile([128, 256], F32, tag="maskP")   # 1 at d==0 else -big
        nc.gpsimd.memset(maskP[:], -3e38)
        nc.gpsimd.memset(
            maskP[:].rearrange("p (g d) -> p g d", d=32)[:, :, 0:1], 1.0)

        halfpi = pers.tile([128, 1], F32, tag="halfpi")
        nc.gpsimd.memset(halfpi[:], float(np.pi / 2))

        # persistent activations
        xt = [pers.tile([128, N], BF16, tag=f"xt{dc}", name=f"xt{dc}")
              for dc in range(DCH)]
        kt = pers.tile([128, N], BF16, tag="kt")
        # vb padded: per jc block of 256 cols: [v_h0(64) | 0(64) | 0(64) | v_h1(64)]
        vbp = pers.tile([128, 2 * N], BF16, tag="vbp")
        nc.gpsimd.memset(vbp[:], 0.0)
        graw = pers.tile([128, N], BF16, tag="graw")
        acrT = pers.tile([128, N], F32, tag="acrT")
        krT = pers.tile([128, N], F32, tag="krT")

        # weights
        wq_t = pers.tile([128, D], BF16, tag="wq_t")
        wk_t = pers.tile([128, D], BF16, tag="wk_t")
        wg_t = pers.tile([128, D], BF16, tag="wg_t")
        wv_t = pers.tile([128, D], BF16, tag="wv_t")
        wo_t = pers.tile([128, D], BF16, tag="wo_t")
        for wt, WT in ((wq_t, WQ), (wk_t, WK), (wg_t, WG), (wv_t, WV)):
            nc.sync.dma_start(
                wt[:].rearrange("p (dc c) -> p dc c", dc=DCH),
                WT[:].rearrange("(dc p) c -> p dc c", p=128))
        nc.sync.dma_start(wo_t[:], WO[:])
        wq_sb = [wq_t[:, dc * 128:(dc + 1) * 128] for dc in range(DCH)]
        wk_sb = [wk_t[:, dc * 128:(dc + 1) * 128] for dc in range(DCH)]
        wg_sb = [wg_t[:, dc * 128:(dc + 1) * 128] for dc in range(DCH)]
        wv_sb = [wv_t[:, dc * 128:(dc + 1) * 128] for dc in range(DCH)]
        wo_sb = [wo_t[:, ji * 128:(ji + 1) * 128] for ji in range(DCH)]
        wa_sb = [pers.tile([128, 128], F32, tag=f"wa{dc}", name=f"wa{dc}")
                 for dc in range(DCH)]
        for dc in range(DCH):
            nc.sync.dma_start(wa_sb[dc][:], WA[dc * 128:(dc + 1) * 128, :])

        # ---- PE warmup: keep HAM busy while startup DMAs land ----
        for w in range(N_WARM):
            pw = ps.tile([128, 128], F32, tag="pout0" if w % 2 == 0 else "acc",
                         name="pw", bufs=1 if w % 2 == 0 else 2)
            nc.tensor.matmul(pw[:], ident[:], ident[:], start=True, stop=True)

        state = {}

        def stage1(c4):
            """x natural load -> exact fp32 transpose -> xb (fp32, a-proj)
            + xt (bf16, projections); fp32 a-projection; a back to natural."""
            ns = slice(c4 * 512, (c4 + 1) * 512)
            xn4 = []
            for i in range(4):
                xn = xnat.tile([128, D], F32, tag="xn", name="xn")
                nci = c4 * 4 + i
                nc.sync.dma_start(xn[:], X[nci * 128:(nci + 1) * 128, :])
                xn4.append(xn)
            xb4 = []
            pa4 = ps.tile([128, 512], F32, tag="acc", bufs=2)
            for dc in range(DCH):
                pt = ps.tile([128, 512], F32, tag="ptr")
                for i in range(4):
                    nc.tensor.transpose(
                        pt[:, i * 128:(i + 1) * 128],
                        xn4[i][:, dc * 128:(dc + 1) * 128], ident[:])
                xb = aep.tile([128, 512], F32, tag="xb", name="xb", bufs=3)
                eng = nc.scalar if dc % 2 == 0 else nc.vector
                if dc % 2 == 0:
                    nc.scalar.copy(xb[:], pt[:])
                else:
                    nc.vector.tensor_copy(xb[:], pt[:])
                nc.gpsimd.tensor_copy(xt[dc][:, ns], xb[:])  # bf16 cast
                xb4.append(xb)
                if dc > 0:  # a-proj MM lags one chunk: keeps PE dense
                    nc.tensor.matmul(pa4[:], wa_sb[dc - 1][:], xb4[dc - 1][:],
                                     start=(dc == 1), stop=False)
            nc.tensor.matmul(pa4[:], wa_sb[DCH - 1][:], xb4[DCH - 1][:],
                             start=False, stop=True)
            at_sb = aep.tile([128, 512], F32, tag="xb", name="at_sb", bufs=3)
            nc.scalar.copy(at_sb[:], pa4[:])
            # transpose aT back to natural [n, (s c h d)]
            pan = ps.tile([128, 512], F32, tag="ptr")
            for s in range(4):
                nc.tensor.transpose(
                    pan[:, s * 128:(s + 1) * 128],
                    at_sb[:, s * 128:(s + 1) * 128], ident[:])
            panS = ach.tile([128, 512], F32, tag="panS", name="panS")
            nc.vector.tensor_copy(panS[:], pan[:])
            state[c4] = panS

        def stageVG(c4):
            """v projection + bf16 transpose into padded vbp; g projection."""
            ns = slice(c4 * 512, (c4 + 1) * 512)
            pv = ps.tile([128, 512], F32, tag="acc", bufs=2)
            for dc in range(DCH):
                nc.tensor.matmul(pv[:], wv_sb[dc], xt[dc][:, ns],
                                 start=(dc == 0), stop=(dc == DCH - 1))
            vtile = gte.tile([128, 512], BF16, tag="vt", name="vt")
            nc.vector.tensor_copy(vtile[:], pv[:])
            pvn = ps.tile([128, 512], BF16, tag="ptr", name="pvn")
            for s in range(4):
                nc.tensor.transpose(
                    pvn[:, s * 128:(s + 1) * 128],
                    vtile[:, s * 128:(s + 1) * 128], identb[:])
            # scatter into padded vbp: jc = 4*c4 + s
            # h0 -> cols jc*256 + [0:64); h1 -> cols jc*256 + [192:256)
            src = pvn[:].rearrange("p (s hd) -> p s hd", s=4)
            dst = vbp[:, c4 * 1024:(c4 + 1) * 1024].rearrange(
                "p (s q) -> p s q", s=4)
            nc.vector.tensor_copy(dst[:, :, 0:64], src[:, :, 0:64])
            nc.vector.tensor_copy(dst[:, :, 192:256], src[:, :, 64:128])
            pg = ps.tile([128, 512], F32, tag="acc", bufs=2)
            for dc in range(DCH):
                nc.tensor.matmul(pg[:], wg_sb[dc], xt[dc][:, ns],
                                 start=(dc == 0), stop=(dc == DCH - 1))
            nc.scalar.copy(graw[:, ns], pg[:])  # bf16 cast via Copy

        def stageA(c4):
            """polar a-chain: mag/phase -> segmented scans -> acr, 1/acr."""
            ns = slice(c4 * 512, (c4 + 1) * 512)
            panS = state.pop(c4)
            pv5 = panS[:].rearrange("p (s c h d) -> p s c h d", s=4, c=2, h=2)
            re, im = pv5[:, :, 0], pv5[:, :, 1]       # [128, 4, 2, 32]
            t1 = ach.tile([128, 256], F32, tag="t1", name="t1")
            t2 = ach.tile([128, 256], F32, tag="t2", name="t2")
            t3 = ach.tile([128, 256], F32, tag="t3", name="t3")
            v1 = t1[:].rearrange("p (s h d) -> p s h d", s=4, h=2)
            v2 = t2[:].rearrange("p (s h d) -> p s h d", s=4, h=2)
            v3 = t3[:].rearrange("p (s h d) -> p s h d", s=4, h=2)
            # magnitude^2
            nc.vector.tensor_mul(v1, re, re)
            nc.vector.tensor_mul(v2, im, im)
            nc.vector.tensor_add(t1[:], t1[:], t2[:])          # m2
            mag = ach.tile([128, 256], F32, tag="mag", name="mag")
            nc.scalar.activation(mag[:], t1[:], AF.Sqrt)
            ssig = ach.tile([128, 256], F32, tag="ssig", name="ssig")
            nc.scalar.activation(ssig[:], mag[:], AF.Sigmoid)
            # phase = arctan(im/re) + pi * (re<0) * sign(im)
            pv4 = panS[:].rearrange("p (s c hd) -> p s c hd", s=4, c=2)
            re3, im3 = pv4[:, :, 0], pv4[:, :, 1]              # [128, 4, 64]
            u1 = t1[:].rearrange("p (s hd) -> p s hd", s=4)
            u2 = t2[:].rearrange("p (s hd) -> p s hd", s=4)
            nc.vector.reciprocal_approx_fast(u1, re3)          # 1/re
            nc.vector.tensor_mul(u2, u1, im3)                  # im/re
            at0 = ach.tile([128, 256], F32, tag="at0", name="at0")
            nc.scalar.activation(at0[:], t2[:], AF.Arctan)
            sgn = ach.tile([128, 256], F32, tag="sgn", name="sgn")
            nc.scalar.activation(
                sgn[:].rearrange("p (s h d) -> p s h d", s=4, h=2), im,
                AF.Sign)
            nc.vector.tensor_scalar(v3, re, 0.0, None, ALU.is_lt)  # re<0
            nc.vector.tensor_mul(t3[:], t3[:], sgn[:])
            phi = ach.tile([128, 256], F32, tag="phi", name="phi")
            nc.vector.scalar_tensor_tensor(
                out=phi[:], in0=t3[:], scalar=float(np.pi), in1=at0[:],
                op0=ALU.mult, op1=ALU.add)
            # segmented scans over d
            Phi = ach.tile([128, 256], F32, tag="Phi", name="Phi")
            nc.vector.tensor_tensor_scan(Phi[:], mask0[:], phi[:], 0.0,
                                         ALU.mult, ALU.add)
            Rm = ach.tile([128, 256], F32, tag="Rm", name="Rm")
            nc.vector.tensor_tensor_scan(Rm[:], maskP[:], ssig[:], 0.0,
                                         ALU.max, ALU.mult)
            # cos(Phi) via round + cody-waite + Sin(x + pi/2)
            nc.vector.tensor_scalar(t1[:], Phi[:],
                                    float(1.0 / (2 * np.pi)), 0.25,
                                    ALU.mult, ALU.add)
            nc.vector.tensor_scalar(t2[:], t1[:], MAGIC, MAGIC,
                                    ALU.add, ALU.subtract)
            nc.vector.cody_waite_cascade(t1[:], Phi[:], t2[:], C1, C2, C3)
            # silu for the gates BEFORE Sin so the silu/sin table loads once
            gs = gte.tile([128, 512], BF16, tag="gs", name="gs")
            nc.scalar.activation(gs[:], graw[:, ns], AF.Silu)
            cosv = ach.tile([128, 256], F32, tag="cosv", name="cosv")
            nc.scalar.activation(cosv[:], t1[:], AF.Sin, bias=halfpi[:, 0:1])
            nc.vector.tensor_mul(t3[:], Rm[:], cosv[:])        # acr (unclipped)
            # clip + expand pairs -> acrE [128, (s h d c)]
            acrE = ach.tile([128, 512], F32, tag="acrE", name="acrE")
            ae4 = acrE[:].rearrange("p (s h d c) -> p s h d c", s=4, h=2, c=2)
            nc.vector.tensor_scalar_max(ae4[:, :, :, :, 0], v3, EPS)
            nc.vector.tensor_scalar_max(ae4[:, :, :, :, 1], v3, EPS)
            pae = ps.tile([128, 512], F32, tag="ptr", name="pae")
            for s in range(4):
                nc.tensor.transpose(pae[:, s * 128:(s + 1) * 128],
                                    acrE[:, s * 128:(s + 1) * 128], ident[:])
            nc.scalar.copy(acrT[:, ns], pae[:])
            nc.vector.reciprocal_approx_fast(krT[:, ns], acrT[:, ns])
            state[(c4, "gs")] = gs

        def stageQK(c4):
            """q/k projections (bf16) + decay scaling into bf16 qt/kt."""
            ns = slice(c4 * 512, (c4 + 1) * 512)
            pq = ps.tile([128, 512], F32, tag="acc", bufs=2)
            for dc in range(DCH):
                nc.tensor.matmul(pq[:], wq_sb[dc], xt[dc][:, ns],
                                 start=(dc == 0), stop=(dc == DCH - 1))
            qt = chk.tile([128, 512], BF16, tag="qt", name="qt")
            nc.vector.tensor_mul(qt[:], pq[:], acrT[:, ns])
            pk = ps.tile([128, 512], F32, tag="acc", bufs=2)
            for dc in range(DCH):
                nc.tensor.matmul(pk[:], wk_sb[dc], xt[dc][:, ns],
                                 start=(dc == 0), stop=(dc == DCH - 1))
            nc.vector.tensor_mul(kt[:, ns], pk[:], krT[:, ns])
            state[(c4, "qt")] = qt

        def stageT(c4):
            """row-packed qk, masked ss copies, padded AV, gating, out-proj."""
            ns = slice(c4 * 512, (c4 + 1) * 512)
            qt = state.pop((c4, "qt"))
            gs = state.pop((c4, "gs"))
            njc = 4 * (c4 + 1)
            pouts = ps.tile([128, 512], F32, tag="pout0", name="pouts",
                            bufs=1)

            def av_mm(ss_pair, jc):
                for h in range(H_LOC):
                    nc.tensor.matmul(
                        pouts[:],
                        vbp[:, jc * 256 + h * 128: jc * 256 + h * 128 + 128],
                        ss_pair[h][:],
                        start=(jc == 0 and h == 0), stop=(jc == njc - 1),
                        skip_group_check=True)

            pend = []
            ssi = 0
            for jc in range(njc):
                psims = []
                for h in range(H_LOC):
                    psim = ps.tile([128, 512], F32, tag="psim", name="psim",
                                   bufs=3)
                    hp = slice(h * 64, (h + 1) * 64)
                    nc.tensor.matmul(
                        psim[:], kt[hp, jc * 128:(jc + 1) * 128],
                        qt[hp, :], start=True, stop=True)
                    psims.append(psim)
                off = jc - 4 * c4
                sss = []
                for h in range(H_LOC):
                    ss = sse.tile([128, 512], BF16, tag="ss", name="ss",
                                  bufs=6)
                    if off >= 0:
                        nc.vector.tensor_mul(
                            ss[:], psims[h][:],
                            maskt[:, off * 512:(off + 1) * 512])
                    elif ssi % 3 == 2:
                        nc.scalar.copy(ss[:], psims[h][:])
                    else:
                        nc.vector.tensor_copy(ss[:], psims[h][:])
                    ssi += 1
                    sss.append(ss)
                pend.append((sss, jc))
                if len(pend) > 1:
                    av_mm(*pend.pop(0))
            for it in pend:
                av_mm(*it)
            gt = gte.tile([128, 512], BF16, tag="gt", name="gt")
            nc.vector.tensor_mul(gt[:], pouts[:], gs[:])
            for ji in range(DCH):
                poj = ps.tile([128, 512], F32,
                              tag=("acc" if ji % 2 == 0 else "pout0"),
                              name="poj", bufs=2 if ji % 2 == 0 else 1)
                nc.tensor.matmul(poj[:], wo_sb[ji], gt[:],
                                 start=True, stop=True)
                ot = gte.tile([128, 512], F32, tag="ot", name="ot", bufs=3)
                if ji % 2 == 0:
                    nc.scalar.copy(ot[:], poj[:])
                else:
                    nc.vector.tensor_copy(ot[:], poj[:])
                nc.sync.dma_start(OUT[ji * 128:(ji + 1) * 128, ns], ot[:])

        stage1(0)
        stage1(1)
        stageVG(0)
        stageA(0)
        stage1(2)
        stageVG(1)
        stageA(1)
        stageQK(0)
        stage1(3)
        stageT(0)
        stageVG(2)
        stageA(2)
        stageQK(1)
        stageT(1)
        stageVG(3)
        stageA(3)
        stageQK(2)
        stageT(2)
        stageQK(3)
        stageT(3)
    nc.finalize()
    return nc


_NC_CACHE = []


def _get_nc():
    if not _NC_CACHE:
        nc = bacc.Bacc("TRN2", target_bir_lowering=False, debug=False)
        _emit(nc)
        _NC_CACHE.append(nc)
    return _NC_CACHE[0]


def _to_bf16(a):
    import ml_dtypes
    return np.ascontiguousarray(a.astype(ml_dtypes.bfloat16))


def _shard_inputs(x, W_qkv, W_a, W_g, W_out, b_out):
    x2 = np.ascontiguousarray(np.asarray(x, np.float32).reshape(N, D))
    W_qkv = np.asarray(W_qkv, np.float32)
    W_a = np.asarray(W_a, np.float32)
    W_g = np.asarray(W_g, np.float32)
    W_out = np.asarray(W_out, np.float32)

    # W_a column permutation: within a core's 128 cols, source col
    # h*64 + 2d + c  ->  dest col c*64 + h*32 + d
    perm = np.empty(128, np.int64)
    for c in range(2):
        for h in range(2):
            for d in range(DC):
                perm[c * 64 + h * 32 + d] = h * 64 + 2 * d + c

    in_maps = []
    for r in range(NCORES):
        cs = r * 128
        wq = _to_bf16(W_qkv[:, cs:cs + 128] * np.float32(DH ** -0.5))
        wk = _to_bf16(W_qkv[:, D + cs:D + cs + 128])
        wv = _to_bf16(W_qkv[:, 2 * D + cs:2 * D + cs + 128])
        wa = np.ascontiguousarray(W_a[:, cs:cs + 128][:, perm])
        wg = _to_bf16(W_g[:, cs:cs + 128])
        wo = _to_bf16(W_out[cs:cs + 128, :])
        in_maps.append({
            "X": x2, "WQ": wq, "WK": wk, "WA": wa, "WV": wv, "WG": wg,
            "WO": wo,
        })
    return in_maps


def _unshard(results, b_out):
    outT = np.zeros((D, N), np.float32)
    for r in results:
        outT += r["OUT"]
    outT += np.asarray(b_out, np.float32).reshape(D, 1)
    return np.ascontiguousarray(outT.T).reshape(1, N, D)


def run(trace=False, **inputs):
    nc = _get_nc()
    in_maps = _shard_inputs(**inputs)
    res = run_bass_kernel_spmd(nc, in_maps, core_ids=list(range(NCORES)),
                               trace=trace)
    return _unshard(res.results, inputs["b_out"]), res


def kernel(**inputs) -> np.ndarray:
    out, _ = run(trace=False, **inputs)
    return out


# revision 8
# speedup vs baseline: 1.0180x; 1.0180x over previous
"""Trainium2 Bass kernel for nn_CausalFullAttention_37821482009327.

Causal full attention (no softmax) with data-dependent complex relative
position decay, silu gating, and output projection.

Sharding: tensor-parallel over the 16 heads -> 2 heads per NeuronCore x 8.
Each core computes its heads' attention and a partial out-projection; the
host sums the 8 partials and adds b_out (the "all-reduce" at gather time).

v3 design:
- x and W_a are split hi/lo into bf16 pairs on the host; the precision-
  critical a-projection runs as 3 accumulating bf16 matmuls
  (hi@Whi + lo@Whi + hi@Wlo ~ 2^-16 effective precision) and every
  transpose is a cheap 1-pass bf16 transpose.
- a-chain in polar form, batched per chunk-PAIR ([128,1024] tiles):
  phase via Arctan + segmented cumsum (HW tensor_tensor_scan), magnitude
  via sigmoid + segmented cumprod (max/mult scan), acr = R * cos(Phi)
  with cody-waite range reduction. Gates use x*sigmoid(x) so all ACTs fit
  3 act-tables per pair-session (sqrt / sigmoid+arctan+sign / sin).
- qk pairs row-packed via base_partition 0/64 (concurrent in the PE).
- AV uses zero-padded stationaries so both heads accumulate into one
  full-partition PSUM bank.
- PE warmup matmuls at t=0 against the HAM clock gate.

Shapes (hardcoded): B=1, N=2048, D=1024, H=16, Dh=64, Dc=32.
"""
import sys

sys.path.insert(0, "/opt/trn_rl_repo")

import numpy as np

import concourse.bass as bass
import concourse.tile as tile
from concourse import bacc, mybir
from concourse.bass_utils import run_bass_kernel_spmd
from concourse.masks import make_identity

F32 = mybir.dt.float32
BF16 = mybir.dt.bfloat16
AF = mybir.ActivationFunctionType
ALU = mybir.AluOpType

N = 2048
D = 1024
H_LOC = 2          # heads per core
DH = 64
DC = 32
NCORES = 8
EPS = 1e-10

NCH = N // 128
DCH = D // 128     # 8 d-chunks of 128
NC4 = N // 512     # 4 n-chunks of 512

MAGIC = float(1.5 * 2 ** 23)
# 2*pi = C1 + C2 + C3 cody-waite split
C1 = float(np.float32(6.28125))
C2 = float(np.float32(0.0019302368))
C3 = float(2 * np.pi - 6.28125 - float(np.float32(0.0019302368)))

N_WARM = 20        # PE warmup matmuls (HAM clock-gate)


def _emit(nc):
    XHI = nc.dram_tensor("XHI", [N, D], BF16, kind="ExternalInput")
    XLO = nc.dram_tensor("XLO", [N, D], BF16, kind="ExternalInput")
    WQ = nc.dram_tensor("WQ", [D, 128], BF16, kind="ExternalInput")
    WK = nc.dram_tensor("WK", [D, 128], BF16, kind="ExternalInput")
    WAH = nc.dram_tensor("WAH", [D, 128], BF16, kind="ExternalInput")
    WAL = nc.dram_tensor("WAL", [D, 128], BF16, kind="ExternalInput")
    WV = nc.dram_tensor("WV", [D, 128], BF16, kind="ExternalInput")
    WG = nc.dram_tensor("WG", [D, 128], BF16, kind="ExternalInput")
    WO = nc.dram_tensor("WO", [128, D], BF16, kind="ExternalInput")
    OUT = nc.dram_tensor("OUT", [D, N], F32, kind="ExternalOutput")

    with (
        tile.TileContext(nc) as tc,
        tc.tile_pool(name="pers", bufs=1) as pers,
        tc.tile_pool(name="ps", bufs=2, space="PSUM") as ps,
        tc.tile_pool(name="xnat", bufs=4) as xnat,
        tc.tile_pool(name="xlop", bufs=12) as xlop,
        tc.tile_pool(name="ach", bufs=2) as ach,
        tc.tile_pool(name="sse", bufs=6) as sse,
        tc.tile_pool(name="chk", bufs=2) as chk,
        tc.tile_pool(name="gte", bufs=2) as gte,
    ):
        # ---- identities first, then PE warmers (HAM warm ASAP) ----
        ident = pers.tile([128, 128], F32, tag="ident")
        make_identity(nc, ident[:])
        identb = pers.tile([128, 128], BF16, tag="identb")
        make_identity(nc, identb[:])
        for w in range(N_WARM):
            pw = ps.tile([128, 128], F32, tag="pout0" if w % 2 == 0 else "acc",
                         name="pw", bufs=1 if w % 2 == 0 else 2)
            nc.tensor.matmul(pw[:], ident[:], ident[:], start=True, stop=True)

        # ---- masks / constants ----
        maskt = pers.tile([128, 4 * 512], F32, tag="maskt")
        for off in range(4):
            m = maskt[:, off * 512:(off + 1) * 512]
            nc.gpsimd.memset(m, 1.0)
            nc.gpsimd.affine_select(
                out=m, in_=m, compare_op=ALU.is_ge,
                fill=0.0, base=-128 * off, pattern=[[1, 512]],
                channel_multiplier=-1)
        # scan segment masks, pair-sized (free layout (c4pair, s4, h2, d32))
        mask0 = pers.tile([128, 512], F32, tag="mask0")   # 0 at d==0 else 1
        nc.gpsimd.memset(mask0[:], 1.0)
        nc.gpsimd.memset(
            mask0[:].rearrange("p (g d) -> p g d", d=32)[:, :, 0:1], 0.0)
        maskP = pers.tile([128, 512], F32, tag="maskP")   # 1 at d==0 else -big
        nc.gpsimd.memset(maskP[:], -3e38)
        nc.gpsimd.memset(
            maskP[:].rearrange("p (g d) -> p g d", d=32)[:, :, 0:1], 1.0)
        halfpi = pers.tile([128, 1], F32, tag="halfpi")
        nc.gpsimd.memset(halfpi[:], float(np.pi / 2))

        # persistent activations
        xt = [pers.tile([128, N], BF16, tag=f"xt{dc}", name=f"xt{dc}")
              for dc in range(DCH)]
        kt = pers.tile([128, N], BF16, tag="kt")
        # vb padded: per jc block of 256: [v_h0(64) | 0 | 0 | v_h1(64)]
        vbp = pers.tile([128, 2 * N], BF16, tag="vbp")
        nc.gpsimd.memset(vbp[:], 0.0)
        graw = pers.tile([128, N], BF16, tag="graw")
        gsig = pers.tile([128, N], BF16, tag="gsig")
        gs = pers.tile([128, N], BF16, tag="gs")
        acrT = pers.tile([128, N], F32, tag="acrT")
        krT = pers.tile([128, N], F32, tag="krT")

        # weights (on the scalar DMA queue; x loads own the sync queue)
        wq_t = pers.tile([128, D], BF16, tag="wq_t")
        wk_t = pers.tile([128, D], BF16, tag="wk_t")
        wg_t = pers.tile([128, D], BF16, tag="wg_t")
        wv_t = pers.tile([128, D], BF16, tag="wv_t")
        wah_t = pers.tile([128, D], BF16, tag="wah_t")
        wal_t = pers.tile([128, D], BF16, tag="wal_t")
        wo_t = pers.tile([128, D], BF16, tag="wo_t")
        for wt, WT in ((wq_t, WQ), (wk_t, WK), (wg_t, WG), (wv_t, WV),
                       (wah_t, WAH), (wal_t, WAL)):
            nc.scalar.dma_start(
                wt[:].rearrange("p (dc c) -> p dc c", dc=DCH),
                WT[:].rearrange("(dc p) c -> p dc c", p=128))
        nc.scalar.dma_start(wo_t[:], WO[:])
        wq_sb = [wq_t[:, dc * 128:(dc + 1) * 128] for dc in range(DCH)]
        wk_sb = [wk_t[:, dc * 128:(dc + 1) * 128] for dc in range(DCH)]
        wg_sb = [wg_t[:, dc * 128:(dc + 1) * 128] for dc in range(DCH)]
        wv_sb = [wv_t[:, dc * 128:(dc + 1) * 128] for dc in range(DCH)]
        wah_sb = [wah_t[:, dc * 128:(dc + 1) * 128] for dc in range(DCH)]
        wal_sb = [wal_t[:, dc * 128:(dc + 1) * 128] for dc in range(DCH)]
        wo_sb = [wo_t[:, ji * 128:(ji + 1) * 128] for ji in range(DCH)]

        state = {}

        def stage1(c4):
            """load xhi/xlo chunk, bf16 transposes, 3-term fp32-ish a-proj,
            a back to natural into the pair buffer."""
            ns = slice(c4 * 512, (c4 + 1) * 512)
            xh4, xl4 = [], []
            for i in range(4):
                xh = xnat.tile([128, D], BF16, tag="xh", name="xh")
                xl = xnat.tile([128, D], BF16, tag="xl", name="xl")
                nci = c4 * 4 + i
                nc.sync.dma_start(xh[:], XHI[nci * 128:(nci + 1) * 128, :])
                nc.sync.dma_start(xl[:], XLO[nci * 128:(nci + 1) * 128, :])
                xh4.append(xh)
                xl4.append(xl)
            pa4 = ps.tile([128, 512], F32, tag="acc", bufs=2)
            lo_tiles = []
            prev = None
            for dc in range(DCH):
                pt = ps.tile([128, 1024], BF16, tag="ptr")
                for i in range(4):
                    nc.tensor.transpose(
                        pt[:, i * 128:(i + 1) * 128],
                        xh4[i][:, dc * 128:(dc + 1) * 128], identb[:])
                    nc.tensor.transpose(
                        pt[:, 512 + i * 128:512 + (i + 1) * 128],
                        xl4[i][:, dc * 128:(dc + 1) * 128], identb[:])
                if dc % 2 == 0:
                    nc.scalar.copy(xt[dc][:, ns], pt[:, 0:512])
                else:
                    nc.vector.tensor_copy(xt[dc][:, ns], pt[:, 0:512])
                xlo = xlop.tile([128, 512], BF16, tag="xlo", name="xlo",
                                bufs=12)
                if dc % 2 == 0:
                    nc.vector.tensor_copy(xlo[:], pt[:, 512:1024])
                else:
                    nc.scalar.copy(xlo[:], pt[:, 512:1024])
                lo_tiles.append(xlo)
                if prev is not None:  # a-proj lags one dc: keeps PE dense
                    pdc = prev
                    nc.tensor.matmul(pa4[:], wah_sb[pdc], xt[pdc][:, ns],
                                     start=(pdc == 0), stop=False)
                    nc.tensor.matmul(pa4[:], wah_sb[pdc], lo_tiles[pdc][:],
                                     start=False, stop=False)
                    nc.tensor.matmul(pa4[:], wal_sb[pdc], xt[pdc][:, ns],
                                     start=False, stop=False)
                prev = dc
            pdc = DCH - 1
            nc.tensor.matmul(pa4[:], wah_sb[pdc], xt[pdc][:, ns],
                             start=False, stop=False)
            nc.tensor.matmul(pa4[:], wah_sb[pdc], lo_tiles[pdc][:],
                             start=False, stop=False)
            nc.tensor.matmul(pa4[:], wal_sb[pdc], xt[pdc][:, ns],
                             start=False, stop=True)
            at_sb = ach.tile([128, 512], F32, tag="at_sb", name="at_sb")
            nc.scalar.copy(at_sb[:], pa4[:])
            # transpose aT back to natural [n, (s c h d)] into the pair buf
            pair = c4 // 2
            if (pair, "panS") not in state:
                state[(pair, "panS")] = ach.tile(
                    [128, 1024], F32, tag="panS", name="panS")
            panS = state[(pair, "panS")]
            po = (c4 % 2) * 512
            pan = ps.tile([128, 512], F32, tag="ptr")
            for s in range(4):
                nc.tensor.transpose(
                    pan[:, s * 128:(s + 1) * 128],
                    at_sb[:, s * 128:(s + 1) * 128], ident[:])
            nc.vector.tensor_copy(panS[:, po:po + 512], pan[:])

        def stageVG(c4):
            """v projection + bf16 transpose into padded vbp; g projection."""
            ns = slice(c4 * 512, (c4 + 1) * 512)
            pv = ps.tile([128, 512], F32, tag="acc", bufs=2)
            for dc in range(DCH):
                nc.tensor.matmul(pv[:], wv_sb[dc], xt[dc][:, ns],
                                 start=(dc == 0), stop=(dc == DCH - 1))
            vtile = gte.tile([128, 512], BF16, tag="vt", name="vt")
            nc.vector.tensor_copy(vtile[:], pv[:])
            pvn = ps.tile([128, 512], BF16, tag="ptr", name="pvn")
            for s in range(4):
                nc.tensor.transpose(
                    pvn[:, s * 128:(s + 1) * 128],
                    vtile[:, s * 128:(s + 1) * 128], identb[:])
            src = pvn[:].rearrange("p (s hd) -> p s hd", s=4)
            dst = vbp[:, c4 * 1024:(c4 + 1) * 1024].rearrange(
                "p (s q) -> p s q", s=4)
            nc.vector.tensor_copy(dst[:, :, 0:64], src[:, :, 0:64])
            nc.vector.tensor_copy(dst[:, :, 192:256], src[:, :, 64:128])
            pg = ps.tile([128, 512], F32, tag="acc", bufs=2)
            for dc in range(DCH):
                nc.tensor.matmul(pg[:], wg_sb[dc], xt[dc][:, ns],
                                 start=(dc == 0), stop=(dc == DCH - 1))
            nc.scalar.copy(graw[:, ns], pg[:])  # bf16 cast via Copy

        def stageA(pair):
            """polar a-chain on a chunk-pair [128,1024]: mag/phase ->
            segmented scans -> acr, 1/acr. Also the gate sigmoids (same
            act-table session)."""
            ns = slice(pair * 1024, (pair + 1) * 1024)
            panS = state.pop((pair, "panS"))
            pv5 = panS[:].rearrange("p (s c h d) -> p s c h d", s=8, c=2, h=2)
            re, im = pv5[:, :, 0], pv5[:, :, 1]       # [128, 8, 2, 32]
            pv4 = panS[:].rearrange("p (s c hd) -> p s c hd", s=8, c=2)
            re3, im3 = pv4[:, :, 0], pv4[:, :, 1]     # [128, 8, 64]
            t1 = ach.tile([128, 512], F32, tag="t1", name="t1", bufs=1)
            t2 = ach.tile([128, 512], F32, tag="t2", name="t2", bufs=1)
            t3 = ach.tile([128, 512], F32, tag="t3", name="t3", bufs=1)
            v1 = t1[:].rearrange("p (s h d) -> p s h d", s=8, h=2)
            v2 = t2[:].rearrange("p (s h d) -> p s h d", s=8, h=2)
            v3 = t3[:].rearrange("p (s h d) -> p s h d", s=8, h=2)
            u1 = t1[:].rearrange("p (s hd) -> p s hd", s=8)
            u2 = t2[:].rearrange("p (s hd) -> p s hd", s=8)
            # |a|^2 on gpsimd (sbuf-only engine)
            nc.gpsimd.tensor_mul(v1, re, re)
            nc.gpsimd.tensor_mul(v2, im, im)
            nc.gpsimd.tensor_add(t1[:], t1[:], t2[:])          # m2
            mag = ach.tile([128, 512], F32, tag="mag", name="mag", bufs=1)
            nc.scalar.activation(mag[:], t1[:], AF.Sqrt)
            ssg = ach.tile([128, 512], F32, tag="ssg", name="ssg", bufs=1)
            nc.scalar.activation(ssg[:], mag[:], AF.Sigmoid)
            # gate sigmoids ride the sigmoid table session
            nc.scalar.activation(gsig[:, ns], graw[:, ns], AF.Sigmoid)
            # phase = arctan(im/re) + pi * (re<0) * sign(im)
            nc.vector.reciprocal_approx_fast(u1, re3)          # 1/re
            nc.vector.tensor_mul(u2, u1, im3)                  # im/re
            nc.vector.tensor_scalar(t2[:], t2[:], -1e4, 1e4, ALU.max,
                                    ALU.min)                   # clamp ratio
            at0 = ach.tile([128, 512], F32, tag="at0", name="at0", bufs=1)
            nc.scalar.activation(at0[:], t2[:], AF.Arctan)
            sgn = ach.tile([128, 512], F32, tag="sgn", name="sgn", bufs=1)
            nc.scalar.activation(
                sgn[:].rearrange("p (s h d) -> p s h d", s=8, h=2), im,
                AF.Sign)
            nc.gpsimd.tensor_scalar(v3, re, 0.0, None, ALU.is_lt)  # re<0
            nc.gpsimd.tensor_mul(t3[:], t3[:], sgn[:])
            phi = ach.tile([128, 512], F32, tag="phi", name="phi", bufs=1)
            nc.vector.scalar_tensor_tensor(
                out=phi[:], in0=t3[:], scalar=float(np.pi), in1=at0[:],
                op0=ALU.mult, op1=ALU.add)
            # segmented scans over d
            Phi = ach.tile([128, 512], F32, tag="Phi", name="Phi", bufs=1)
            nc.vector.tensor_tensor_scan(Phi[:], mask0[:], phi[:], 0.0,
                                         ALU.mult, ALU.add)
            Rm = ach.tile([128, 512], F32, tag="Rm", name="Rm", bufs=1)
            nc.vector.tensor_tensor_scan(Rm[:], maskP[:], ssg[:], 0.0,
                                         ALU.max, ALU.mult)
            # cos(Phi) via round + cody-waite + Sin(x + pi/2)
            nc.gpsimd.tensor_scalar(t1[:], Phi[:],
                                    float(1.0 / (2 * np.pi)), 0.25,
                                    ALU.mult, ALU.add)
            nc.gpsimd.tensor_scalar(t2[:], t1[:], MAGIC, MAGIC,
                                    ALU.add, ALU.subtract)
            nc.vector.cody_waite_cascade(t1[:], Phi[:], t2[:], C1, C2, C3)
            cosv = ach.tile([128, 512], F32, tag="cosv", name="cosv", bufs=1)
            nc.scalar.activation(cosv[:], t1[:], AF.Sin, bias=halfpi[:, 0:1])
            nc.vector.tensor_mul(t3[:], Rm[:], cosv[:])        # acr unclipped
            # gates: x * sigmoid(x) on gpsimd (all-sbuf bf16)
            nc.gpsimd.tensor_mul(gs[:, slice(pair * 1024, (pair + 1) * 1024)],
                                 graw[:, ns], gsig[:, ns])
            # clip + expand pairs -> acrE [128, (s h d c)] (gpsimd, sbuf)
            acrE = ach.tile([128, 1024], F32, tag="acrE", name="acrE", bufs=1)
            ae4 = acrE[:].rearrange("p (s h d c) -> p s h d c", s=8, h=2, c=2)
            nc.gpsimd.tensor_scalar_max(ae4[:, :, :, :, 0], v3, EPS)
            nc.gpsimd.tensor_scalar_max(ae4[:, :, :, :, 1], v3, EPS)
            for half in range(2):
                pae = ps.tile([128, 512], F32, tag="ptr", name="pae")
                for s in range(4):
                    nc.tensor.transpose(
                        pae[:, s * 128:(s + 1) * 128],
                        acrE[:, half * 512 + s * 128:half * 512 + (s + 1) * 128],
                        ident[:])
                hs = slice(pair * 1024 + half * 512,
                           pair * 1024 + (half + 1) * 512)
                nc.scalar.copy(acrT[:, hs], pae[:])
            ksl = slice(pair * 1024, (pair + 1) * 1024)
            nc.vector.reciprocal_approx_fast(krT[:, ksl], acrT[:, ksl])

        def stageQK(c4):
            """q/k projections (bf16) + decay scaling into bf16 qt/kt."""
            ns = slice(c4 * 512, (c4 + 1) * 512)
            pq = ps.tile([128, 512], F32, tag="acc", bufs=2)
            for dc in range(DCH):
                nc.tensor.matmul(pq[:], wq_sb[dc], xt[dc][:, ns],
                                 start=(dc == 0), stop=(dc == DCH - 1))
            qt = chk.tile([128, 512], BF16, tag="qt", name="qt")
            nc.vector.tensor_mul(qt[:], pq[:], acrT[:, ns])
            pk = ps.tile([128, 512], F32, tag="acc", bufs=2)
            for dc in range(DCH):
                nc.tensor.matmul(pk[:], wk_sb[dc], xt[dc][:, ns],
                                 start=(dc == 0), stop=(dc == DCH - 1))
            nc.vector.tensor_mul(kt[:, ns], pk[:], krT[:, ns])
            state[(c4, "qt")] = qt

        def stageT(c4):
            """row-packed qk, masked ss copies, padded AV, gating, out-proj."""
            ns = slice(c4 * 512, (c4 + 1) * 512)
            qt = state.pop((c4, "qt"))
            njc = 4 * (c4 + 1)
            pouts = ps.tile([128, 512], F32, tag="pout0", name="pouts",
                            bufs=1)

            def av_mm(ss_pair, jc):
                for h in range(H_LOC):
                    nc.tensor.matmul(
                        pouts[:],
                        vbp[:, jc * 256 + h * 128: jc * 256 + h * 128 + 128],
                        ss_pair[h][:],
                        start=(jc == 0 and h == 0), stop=(jc == njc - 1),
                        skip_group_check=True)

            pend = []
            ssi = 0
            for jc in range(njc):
                psims = []
                for h in range(H_LOC):
                    psim = ps.tile([128, 512], F32, tag="psim", name="psim",
                                   bufs=3)
                    hp = slice(h * 64, (h + 1) * 64)
                    nc.tensor.matmul(
                        psim[:], kt[hp, jc * 128:(jc + 1) * 128],
                        qt[hp, :], start=True, stop=True)
                    psims.append(psim)
                off = jc - 4 * c4
                sss = []
                for h in range(H_LOC):
                    ss = sse.tile([128, 512], BF16, tag="ss", name="ss",
                                  bufs=6)
                    if off >= 0:
                        nc.vector.tensor_mul(
                            ss[:], psims[h][:],
                            maskt[:, off * 512:(off + 1) * 512])
                    elif ssi % 2 == 0:
                        nc.scalar.copy(ss[:], psims[h][:])
                    else:
                        nc.vector.tensor_copy(ss[:], psims[h][:])
                    ssi += 1
                    sss.append(ss)
                pend.append((sss, jc))
                if len(pend) > 1:
                    av_mm(*pend.pop(0))
            for it in pend:
                av_mm(*it)
            gt = gte.tile([128, 512], BF16, tag="gt", name="gt")
            nc.vector.tensor_mul(gt[:], pouts[:], gs[:, ns])
            for ji in range(DCH):
                poj = ps.tile([128, 512], F32,
                              tag=("acc" if ji % 2 == 0 else "pout0"),
                              name="poj", bufs=2 if ji % 2 == 0 else 1)
                nc.tensor.matmul(poj[:], wo_sb[ji], gt[:],
                                 start=True, stop=True)
                ot = gte.tile([128, 512], F32, tag="ot", name="ot", bufs=3)
                if ji % 2 == 0:
                    nc.scalar.copy(ot[:], poj[:])
                else:
                    nc.vector.tensor_copy(ot[:], poj[:])
                nc.sync.dma_start(OUT[ji * 128:(ji + 1) * 128, ns], ot[:])

        stage1(0)
        stage1(1)
        stageVG(0)
        stageVG(1)
        stageA(0)
        stage1(2)
        stageQK(0)
        stageT(0)
        stageVG(2)
        stageQK(1)
        stage1(3)
        stageT(1)
        stageVG(3)
        stageA(1)
        stageQK(2)
        stageT(2)
        stageQK(3)
        stageT(3)
    nc.finalize()
    return nc


_NC_CACHE = []


def _get_nc():
    if not _NC_CACHE:
        nc = bacc.Bacc("TRN2", target_bir_lowering=False, debug=False)
        _emit(nc)
        _NC_CACHE.append(nc)
    return _NC_CACHE[0]


def _bf16_split(a):
    import ml_dtypes
    hi = a.astype(ml_dtypes.bfloat16)
    lo = (a - hi.astype(np.float32)).astype(ml_dtypes.bfloat16)
    return np.ascontiguousarray(hi), np.ascontiguousarray(lo)


def _to_bf16(a):
    import ml_dtypes
    return np.ascontiguousarray(a.astype(ml_dtypes.bfloat16))


def _shard_inputs(x, W_qkv, W_a, W_g, W_out, b_out):
    x2 = np.ascontiguousarray(np.asarray(x, np.float32).reshape(N, D))
    W_qkv = np.asarray(W_qkv, np.float32)
    W_a = np.asarray(W_a, np.float32)
    W_g = np.asarray(W_g, np.float32)
    W_out = np.asarray(W_out, np.float32)

    xhi, xlo = _bf16_split(x2)

    # W_a column permutation: within a core's 128 cols, source col
    # h*64 + 2d + c  ->  dest col c*64 + h*32 + d
    perm = np.empty(128, np.int64)
    for c in range(2):
        for h in range(2):
            for d in range(DC):
                perm[c * 64 + h * 32 + d] = h * 64 + 2 * d + c

    in_maps = []
    for r in range(NCORES):
        cs = r * 128
        wq = _to_bf16(W_qkv[:, cs:cs + 128] * np.float32(DH ** -0.5))
        wk = _to_bf16(W_qkv[:, D + cs:D + cs + 128])
        wv = _to_bf16(W_qkv[:, 2 * D + cs:2 * D + cs + 128])
        wah, wal = _bf16_split(
            np.ascontiguousarray(W_a[:, cs:cs + 128][:, perm]))
        wg = _to_bf16(W_g[:, cs:cs + 128])
        wo = _to_bf16(W_out[cs:cs + 128, :])
        in_maps.append({
            "XHI": xhi, "XLO": xlo, "WQ": wq, "WK": wk, "WAH": wah,
            "WAL": wal, "WV": wv, "WG": wg, "WO": wo,
        })
    return in_maps


def _unshard(results, b_out):
    outT = np.zeros((D, N), np.float32)
    for r in results:
        outT += r["OUT"]
    outT += np.asarray(b_out, np.float32).reshape(D, 1)
    return np.ascontiguousarray(outT.T).reshape(1, N, D)


def run(trace=False, **inputs):
    nc = _get_nc()
    in_maps = _shard_inputs(**inputs)
    res = run_bass_kernel_spmd(nc, in_maps, core_ids=list(range(NCORES)),
                               trace=trace)
    return _unshard(res.results, inputs["b_out"]), res


def kernel(**inputs) -> np.ndarray:
    out, _ = run(trace=False, **inputs)
    return out


# revision 11
# speedup vs baseline: 1.2395x; 1.2175x over previous
"""Trainium2 Bass kernel for nn_CausalFullAttention_37821482009327.

Causal full attention (no softmax) with data-dependent complex relative
position decay, silu gating, and output projection.

Sharding: tensor-parallel over the 16 heads -> 2 heads per NeuronCore x 8.
Each core computes its heads' attention and a partial out-projection; the
host sums the 8 partials and adds b_out (the "all-reduce" at gather time).

v3 design:
- x and W_a are split hi/lo into bf16 pairs on the host; the precision-
  critical a-projection runs as 3 accumulating bf16 matmuls
  (hi@Whi + lo@Whi + hi@Wlo ~ 2^-16 effective precision) and every
  transpose is a cheap 1-pass bf16 transpose.
- a-chain in polar form, batched per chunk-PAIR ([128,1024] tiles):
  phase via Arctan + segmented cumsum (HW tensor_tensor_scan), magnitude
  via sigmoid + segmented cumprod (max/mult scan), acr = R * cos(Phi)
  with cody-waite range reduction. Gates use x*sigmoid(x) so all ACTs fit
  3 act-tables per pair-session (sqrt / sigmoid+arctan+sign / sin).
- qk pairs row-packed via base_partition 0/64 (concurrent in the PE).
- AV uses zero-padded stationaries so both heads accumulate into one
  full-partition PSUM bank.
- PE warmup matmuls at t=0 against the HAM clock gate.

Shapes (hardcoded): B=1, N=2048, D=1024, H=16, Dh=64, Dc=32.
"""
import sys

sys.path.insert(0, "/opt/trn_rl_repo")

import numpy as np

import concourse.bass as bass
import concourse.tile as tile
from concourse import bacc, mybir
from concourse.bass_utils import run_bass_kernel_spmd

F32 = mybir.dt.float32
BF16 = mybir.dt.bfloat16
AF = mybir.ActivationFunctionType
ALU = mybir.AluOpType

N = 2048
D = 1024
H_LOC = 2          # heads per core
DH = 64
DC = 32
NCORES = 8
EPS = 1e-10

NCH = N // 128
DCH = D // 128     # 8 d-chunks of 128
NC4 = N // 512     # 4 n-chunks of 512

MAGIC = float(1.5 * 2 ** 23)
# 2*pi = C1 + C2 + C3 cody-waite split
C1 = float(np.float32(6.28125))
C2 = float(np.float32(0.0019302368))
C3 = float(2 * np.pi - 6.28125 - float(np.float32(0.0019302368)))

N_WARM = 20        # PE warmup matmuls (HAM clock-gate)


def _emit(nc):
    XHI = nc.dram_tensor("XHI", [N, D], BF16, kind="ExternalInput")
    XLO = nc.dram_tensor("XLO", [N, D], BF16, kind="ExternalInput")
    WQ = nc.dram_tensor("WQ", [D, 128], BF16, kind="ExternalInput")
    WK = nc.dram_tensor("WK", [D, 128], BF16, kind="ExternalInput")
    WAH = nc.dram_tensor("WAH", [D, 128], BF16, kind="ExternalInput")
    WAL = nc.dram_tensor("WAL", [D, 128], BF16, kind="ExternalInput")
    WV = nc.dram_tensor("WV", [D, 128], BF16, kind="ExternalInput")
    WG = nc.dram_tensor("WG", [D, 128], BF16, kind="ExternalInput")
    WO = nc.dram_tensor("WO", [128, D], BF16, kind="ExternalInput")
    IDF = nc.dram_tensor("IDF", [128, 128], F32, kind="ExternalInput")
    IDB = nc.dram_tensor("IDB", [128, 128], BF16, kind="ExternalInput")
    OUT = nc.dram_tensor("OUT", [D, N], F32, kind="ExternalOutput")

    with (
        tile.TileContext(nc) as tc,
        tc.tile_pool(name="pers", bufs=1) as pers,
        tc.tile_pool(name="ps", bufs=2, space="PSUM") as ps,
        tc.tile_pool(name="xnat", bufs=4) as xnat,
        tc.tile_pool(name="xlop", bufs=12) as xlop,
        tc.tile_pool(name="ach", bufs=2) as ach,
        tc.tile_pool(name="sse", bufs=6) as sse,
        tc.tile_pool(name="chk", bufs=2) as chk,
        tc.tile_pool(name="gte", bufs=2) as gte,
    ):
        # ---- identities first (via DMA), then PE warmers (HAM warm ASAP) ----
        ident = pers.tile([128, 128], F32, tag="ident")
        nc.sync.dma_start(ident[:], IDF[:])
        identb = pers.tile([128, 128], BF16, tag="identb")
        nc.sync.dma_start(identb[:], IDB[:])
        for w in range(N_WARM):
            pw = ps.tile([128, 128], F32, tag="pout0" if w % 2 == 0 else "acc",
                         name="pw", bufs=1 if w % 2 == 0 else 2)
            nc.tensor.matmul(pw[:], ident[:], ident[:], start=True, stop=True)

        # ---- masks / constants ----
        maskt = pers.tile([128, 4 * 512], F32, tag="maskt")
        for off in range(4):
            m = maskt[:, off * 512:(off + 1) * 512]
            nc.gpsimd.memset(m, 1.0)
            nc.gpsimd.affine_select(
                out=m, in_=m, compare_op=ALU.is_ge,
                fill=0.0, base=-128 * off, pattern=[[1, 512]],
                channel_multiplier=-1)
        # scan segment masks, pair-sized (free layout (c4pair, s4, h2, d32))
        mask0 = pers.tile([128, 512], F32, tag="mask0")   # 0 at d==0 else 1
        nc.gpsimd.memset(mask0[:], 1.0)
        nc.gpsimd.memset(
            mask0[:].rearrange("p (g d) -> p g d", d=32)[:, :, 0:1], 0.0)
        maskP = pers.tile([128, 512], F32, tag="maskP")   # 1 at d==0 else -big
        nc.gpsimd.memset(maskP[:], -3e38)
        nc.gpsimd.memset(
            maskP[:].rearrange("p (g d) -> p g d", d=32)[:, :, 0:1], 1.0)
        halfpi = pers.tile([128, 1], F32, tag="halfpi")
        nc.gpsimd.memset(halfpi[:], float(np.pi / 2))
        quarter = pers.tile([128, 1], F32, tag="quarter")
        nc.gpsimd.memset(quarter[:], 0.25)
        magicP = pers.tile([128, 1], F32, tag="magicP")
        nc.gpsimd.memset(magicP[:], MAGIC)
        magicN = pers.tile([128, 1], F32, tag="magicN")
        nc.gpsimd.memset(magicN[:], -MAGIC)

        # persistent activations
        xt = [pers.tile([128, N], BF16, tag=f"xt{dc}", name=f"xt{dc}")
              for dc in range(DCH)]
        kt = pers.tile([128, N], BF16, tag="kt")
        # vb padded: per jc block of 256: [v_h0(64) | 0 | 0 | v_h1(64)]
        vbp = pers.tile([128, 2 * N], BF16, tag="vbp")
        nc.gpsimd.memset(vbp[:], 0.0)
        graw = pers.tile([128, N], BF16, tag="graw")
        gsig = pers.tile([128, N], BF16, tag="gsig")
        gs = pers.tile([128, N], BF16, tag="gs")
        acrT = pers.tile([128, N], F32, tag="acrT")
        krT = pers.tile([128, N], F32, tag="krT")

        # weights (on the scalar DMA queue; x loads own the sync queue)
        wq_t = pers.tile([128, D], BF16, tag="wq_t")
        wk_t = pers.tile([128, D], BF16, tag="wk_t")
        wg_t = pers.tile([128, D], BF16, tag="wg_t")
        wv_t = pers.tile([128, D], BF16, tag="wv_t")
        wah_t = pers.tile([128, D], BF16, tag="wah_t")
        wal_t = pers.tile([128, D], BF16, tag="wal_t")
        wo_t = pers.tile([128, D], BF16, tag="wo_t")
        for wt, WT in ((wq_t, WQ), (wk_t, WK), (wg_t, WG), (wv_t, WV),
                       (wah_t, WAH), (wal_t, WAL)):
            nc.scalar.dma_start(
                wt[:].rearrange("p (dc c) -> p dc c", dc=DCH),
                WT[:].rearrange("(dc p) c -> p dc c", p=128))
        nc.scalar.dma_start(wo_t[:], WO[:])
        wq_sb = [wq_t[:, dc * 128:(dc + 1) * 128] for dc in range(DCH)]
        wk_sb = [wk_t[:, dc * 128:(dc + 1) * 128] for dc in range(DCH)]
        wg_sb = [wg_t[:, dc * 128:(dc + 1) * 128] for dc in range(DCH)]
        wv_sb = [wv_t[:, dc * 128:(dc + 1) * 128] for dc in range(DCH)]
        wah_sb = [wah_t[:, dc * 128:(dc + 1) * 128] for dc in range(DCH)]
        wal_sb = [wal_t[:, dc * 128:(dc + 1) * 128] for dc in range(DCH)]
        wo_sb = [wo_t[:, ji * 128:(ji + 1) * 128] for ji in range(DCH)]

        state = {}

        def stage1(c4):
            """load xhi/xlo chunk, bf16 transposes, 3-term fp32-ish a-proj,
            a back to natural into the pair buffer."""
            ns = slice(c4 * 512, (c4 + 1) * 512)
            xh4, xl4 = [], []
            for i in range(4):
                xh = xnat.tile([128, D], BF16, tag="xh", name="xh")
                xl = xnat.tile([128, D], BF16, tag="xl", name="xl")
                nci = c4 * 4 + i
                nc.sync.dma_start(xh[:], XHI[nci * 128:(nci + 1) * 128, :])
                nc.sync.dma_start(xl[:], XLO[nci * 128:(nci + 1) * 128, :])
                xh4.append(xh)
                xl4.append(xl)
            pa4 = ps.tile([128, 512], F32, tag="acc", bufs=2)
            lo_tiles = []
            prev = None
            for dc in range(DCH):
                pt = ps.tile([128, 1024], BF16, tag="ptr")
                for i in range(4):
                    nc.tensor.transpose(
                        pt[:, i * 128:(i + 1) * 128],
                        xh4[i][:, dc * 128:(dc + 1) * 128], identb[:])
                    nc.tensor.transpose(
                        pt[:, 512 + i * 128:512 + (i + 1) * 128],
                        xl4[i][:, dc * 128:(dc + 1) * 128], identb[:])
                if dc % 2 == 0:
                    nc.scalar.copy(xt[dc][:, ns], pt[:, 0:512])
                else:
                    nc.vector.tensor_copy(xt[dc][:, ns], pt[:, 0:512])
                xlo = xlop.tile([128, 512], BF16, tag="xlo", name="xlo",
                                bufs=12)
                if dc % 2 == 0:
                    nc.vector.tensor_copy(xlo[:], pt[:, 512:1024])
                else:
                    nc.scalar.copy(xlo[:], pt[:, 512:1024])
                lo_tiles.append(xlo)
                if prev is not None:  # a-proj lags one dc: keeps PE dense
                    pdc = prev
                    nc.tensor.matmul(pa4[:], wah_sb[pdc], xt[pdc][:, ns],
                                     start=(pdc == 0), stop=False)
                    nc.tensor.matmul(pa4[:], wah_sb[pdc], lo_tiles[pdc][:],
                                     start=False, stop=False)
                    nc.tensor.matmul(pa4[:], wal_sb[pdc], xt[pdc][:, ns],
                                     start=False, stop=False)
                prev = dc
            pdc = DCH - 1
            nc.tensor.matmul(pa4[:], wah_sb[pdc], xt[pdc][:, ns],
                             start=False, stop=False)
            nc.tensor.matmul(pa4[:], wah_sb[pdc], lo_tiles[pdc][:],
                             start=False, stop=False)
            nc.tensor.matmul(pa4[:], wal_sb[pdc], xt[pdc][:, ns],
                             start=False, stop=True)
            at_sb = ach.tile([128, 512], F32, tag="at_sb", name="at_sb")
            nc.scalar.copy(at_sb[:], pa4[:])
            # transpose aT back to natural [n, (s c h d)] into the pair buf
            pair = c4 // 2
            if (pair, "panS") not in state:
                state[(pair, "panS")] = ach.tile(
                    [128, 1024], F32, tag="panS", name="panS")
            panS = state[(pair, "panS")]
            po = (c4 % 2) * 512
            pan = ps.tile([128, 512], F32, tag="ptr")
            for s in range(4):
                nc.tensor.transpose(
                    pan[:, s * 128:(s + 1) * 128],
                    at_sb[:, s * 128:(s + 1) * 128], ident[:])
            nc.vector.tensor_copy(panS[:, po:po + 512], pan[:])

        def stageVG(c4):
            """v projection + bf16 transpose into padded vbp; g projection."""
            ns = slice(c4 * 512, (c4 + 1) * 512)
            pv = ps.tile([128, 512], F32, tag="acc", bufs=2)
            for dc in range(DCH):
                nc.tensor.matmul(pv[:], wv_sb[dc], xt[dc][:, ns],
                                 start=(dc == 0), stop=(dc == DCH - 1))
            vtile = gte.tile([128, 512], BF16, tag="vt", name="vt")
            nc.vector.tensor_copy(vtile[:], pv[:])
            pvn = ps.tile([128, 512], BF16, tag="ptr", name="pvn")
            for s in range(4):
                nc.tensor.transpose(
                    pvn[:, s * 128:(s + 1) * 128],
                    vtile[:, s * 128:(s + 1) * 128], identb[:])
            src = pvn[:].rearrange("p (s hd) -> p s hd", s=4)
            dst = vbp[:, c4 * 1024:(c4 + 1) * 1024].rearrange(
                "p (s q) -> p s q", s=4)
            nc.vector.tensor_copy(dst[:, :, 0:64], src[:, :, 0:64])
            nc.vector.tensor_copy(dst[:, :, 192:256], src[:, :, 64:128])
            pg = ps.tile([128, 512], F32, tag="acc", bufs=2)
            for dc in range(DCH):
                nc.tensor.matmul(pg[:], wg_sb[dc], xt[dc][:, ns],
                                 start=(dc == 0), stop=(dc == DCH - 1))
            nc.scalar.copy(graw[:, ns], pg[:])  # bf16 cast via Copy

        def stageA(pair):
            """polar a-chain on a chunk-pair [128,1024]: mag/phase ->
            segmented scans -> acr, 1/acr. Also the gate sigmoids (same
            act-table session)."""
            ns = slice(pair * 1024, (pair + 1) * 1024)
            panS = state.pop((pair, "panS"))
            pv5 = panS[:].rearrange("p (s c h d) -> p s c h d", s=8, c=2, h=2)
            re, im = pv5[:, :, 0], pv5[:, :, 1]       # [128, 8, 2, 32]
            pv4 = panS[:].rearrange("p (s c hd) -> p s c hd", s=8, c=2)
            re3, im3 = pv4[:, :, 0], pv4[:, :, 1]     # [128, 8, 64]
            t1 = ach.tile([128, 512], F32, tag="t1", name="t1", bufs=1)
            t2 = ach.tile([128, 512], F32, tag="t2", name="t2", bufs=1)
            t3 = ach.tile([128, 512], F32, tag="t3", name="t3", bufs=1)
            v1 = t1[:].rearrange("p (s h d) -> p s h d", s=8, h=2)
            v2 = t2[:].rearrange("p (s h d) -> p s h d", s=8, h=2)
            v3 = t3[:].rearrange("p (s h d) -> p s h d", s=8, h=2)
            u1 = t1[:].rearrange("p (s hd) -> p s hd", s=8)
            u2 = t2[:].rearrange("p (s hd) -> p s hd", s=8)
            nc.vector.tensor_mul(v1, re, re)
            nc.vector.tensor_mul(v2, im, im)
            nc.vector.tensor_add(t1[:], t1[:], t2[:])          # m2
            mag = ach.tile([128, 512], F32, tag="mag", name="mag", bufs=1)
            nc.scalar.activation(mag[:], t1[:], AF.Sqrt)
            ssg = ach.tile([128, 512], F32, tag="ssg", name="ssg", bufs=1)
            nc.scalar.activation(ssg[:], mag[:], AF.Sigmoid)
            # gate sigmoids ride the sigmoid table session
            nc.scalar.activation(gsig[:, ns], graw[:, ns], AF.Sigmoid)
            # phase = arctan(im/re) + pi * (re<0) * sign(im)
            nc.vector.reciprocal_approx_fast(u1, re3)          # 1/re
            nc.vector.tensor_mul(u2, u1, im3)                  # im/re
            nc.vector.tensor_scalar(t2[:], t2[:], -1e4, 1e4, ALU.max,
                                    ALU.min)                   # clamp ratio
            at0 = ach.tile([128, 512], F32, tag="at0", name="at0", bufs=1)
            nc.scalar.activation(at0[:], t2[:], AF.Arctan)
            sgn = ach.tile([128, 512], F32, tag="sgn", name="sgn", bufs=1)
            nc.scalar.activation(
                sgn[:].rearrange("p (s h d) -> p s h d", s=8, h=2), im,
                AF.Sign)
            # pi*(re<0) = pi/2 - pi/2*sign(re)
            sgr = ach.tile([128, 512], F32, tag="sgr", name="sgr", bufs=1)
            nc.scalar.activation(
                sgr[:].rearrange("p (s h d) -> p s h d", s=8, h=2), re,
                AF.Sign)
            nc.scalar.activation(t3[:], sgr[:], AF.Identity,
                                 bias=halfpi[:, 0:1], scale=float(-np.pi / 2))
            nc.vector.tensor_mul(t3[:], t3[:], sgn[:])
            phi = ach.tile([128, 512], F32, tag="phi", name="phi", bufs=1)
            nc.vector.tensor_add(phi[:], t3[:], at0[:])
            # segmented scans over d
            Phi = ach.tile([128, 512], F32, tag="Phi", name="Phi", bufs=1)
            nc.vector.tensor_tensor_scan(Phi[:], mask0[:], phi[:], 0.0,
                                         ALU.mult, ALU.add)
            Rm = ach.tile([128, 512], F32, tag="Rm", name="Rm", bufs=1)
            nc.vector.tensor_tensor_scan(Rm[:], maskP[:], ssg[:], 0.0,
                                         ALU.max, ALU.mult)
            # cos(Phi) via round + cody-waite + Sin(x + pi/2)
            nc.scalar.activation(t1[:], Phi[:], AF.Identity,
                                 bias=quarter[:, 0:1],
                                 scale=float(1.0 / (2 * np.pi)))
            nc.scalar.activation(t2[:], t1[:], AF.Identity,
                                 bias=magicP[:, 0:1])
            nc.scalar.activation(t2[:], t2[:], AF.Identity,
                                 bias=magicN[:, 0:1])
            nc.vector.cody_waite_cascade(t1[:], Phi[:], t2[:], C1, C2, C3)
            cosv = ach.tile([128, 512], F32, tag="cosv", name="cosv", bufs=1)
            nc.scalar.activation(cosv[:], t1[:], AF.Sin, bias=halfpi[:, 0:1])
            nc.vector.tensor_mul(t3[:], Rm[:], cosv[:])        # acr unclipped
            # gates: x * sigmoid(x) on gpsimd (all-sbuf bf16)
            nc.gpsimd.tensor_mul(gs[:, slice(pair * 1024, (pair + 1) * 1024)],
                                 graw[:, ns], gsig[:, ns])
            # clip + expand pairs -> acrE [128, (s h d c)] (gpsimd, sbuf)
            acrE = ach.tile([128, 1024], F32, tag="acrE", name="acrE", bufs=1)
            ae4 = acrE[:].rearrange("p (s h d c) -> p s h d c", s=8, h=2, c=2)
            nc.vector.tensor_scalar_max(ae4[:, :, :, :, 0], v3, EPS)
            nc.vector.tensor_scalar_max(ae4[:, :, :, :, 1], v3, EPS)
            for half in range(2):
                pae = ps.tile([128, 512], F32, tag="ptr", name="pae")
                for s in range(4):
                    nc.tensor.transpose(
                        pae[:, s * 128:(s + 1) * 128],
                        acrE[:, half * 512 + s * 128:half * 512 + (s + 1) * 128],
                        ident[:])
                hs = slice(pair * 1024 + half * 512,
                           pair * 1024 + (half + 1) * 512)
                nc.scalar.copy(acrT[:, hs], pae[:])
            ksl = slice(pair * 1024, (pair + 1) * 1024)
            nc.vector.reciprocal_approx_fast(krT[:, ksl], acrT[:, ksl])

        def stageQK(c4):
            """q/k projections (bf16) + decay scaling into bf16 qt/kt."""
            ns = slice(c4 * 512, (c4 + 1) * 512)
            pq = ps.tile([128, 512], F32, tag="acc", bufs=2)
            for dc in range(DCH):
                nc.tensor.matmul(pq[:], wq_sb[dc], xt[dc][:, ns],
                                 start=(dc == 0), stop=(dc == DCH - 1))
            qt = chk.tile([128, 512], BF16, tag="qt", name="qt")
            nc.vector.tensor_mul(qt[:], pq[:], acrT[:, ns])
            pk = ps.tile([128, 512], F32, tag="acc", bufs=2)
            for dc in range(DCH):
                nc.tensor.matmul(pk[:], wk_sb[dc], xt[dc][:, ns],
                                 start=(dc == 0), stop=(dc == DCH - 1))
            nc.vector.tensor_mul(kt[:, ns], pk[:], krT[:, ns])
            state[(c4, "qt")] = qt

        def stageT(c4):
            """row-packed qk, masked ss copies, padded AV, gating, out-proj."""
            ns = slice(c4 * 512, (c4 + 1) * 512)
            qt = state.pop((c4, "qt"))
            njc = 4 * (c4 + 1)
            pouts = ps.tile([128, 512], F32, tag="pout0", name="pouts",
                            bufs=1)

            def av_mm(ss_pair, jc):
                for h in range(H_LOC):
                    nc.tensor.matmul(
                        pouts[:],
                        vbp[:, jc * 256 + h * 128: jc * 256 + h * 128 + 128],
                        ss_pair[h][:],
                        start=(jc == 0 and h == 0), stop=(jc == njc - 1),
                        skip_group_check=True)

            pend = []
            ssi = 0
            for jc in range(njc):
                psims = []
                for h in range(H_LOC):
                    psim = ps.tile([128, 512], F32, tag="psim", name="psim",
                                   bufs=3)
                    hp = slice(h * 64, (h + 1) * 64)
                    nc.tensor.matmul(
                        psim[:], kt[hp, jc * 128:(jc + 1) * 128],
                        qt[hp, :], start=True, stop=True)
                    psims.append(psim)
                off = jc - 4 * c4
                sss = []
                for h in range(H_LOC):
                    ss = sse.tile([128, 512], BF16, tag="ss", name="ss",
                                  bufs=6)
                    if off >= 0:
                        nc.vector.tensor_mul(
                            ss[:], psims[h][:],
                            maskt[:, off * 512:(off + 1) * 512])
                    elif ssi % 2 == 0:
                        nc.scalar.copy(ss[:], psims[h][:])
                    else:
                        nc.vector.tensor_copy(ss[:], psims[h][:])
                    ssi += 1
                    sss.append(ss)
                pend.append((sss, jc))
                if len(pend) > 1:
                    av_mm(*pend.pop(0))
            for it in pend:
                av_mm(*it)
            gt = gte.tile([128, 512], BF16, tag="gt", name="gt")
            nc.vector.tensor_mul(gt[:], pouts[:], gs[:, ns])
            for ji in range(DCH):
                poj = ps.tile([128, 512], F32,
                              tag=("acc" if ji % 2 == 0 else "pout0"),
                              name="poj", bufs=2 if ji % 2 == 0 else 1)
                nc.tensor.matmul(poj[:], wo_sb[ji], gt[:],
                                 start=True, stop=True)
                ot = gte.tile([128, 512], F32, tag="ot", name="ot", bufs=3)
                if ji % 2 == 0:
                    nc.scalar.copy(ot[:], poj[:])
                else:
                    nc.vector.tensor_copy(ot[:], poj[:])
                nc.sync.dma_start(OUT[ji * 128:(ji + 1) * 128, ns], ot[:])

        stage1(0)
        stage1(1)
        stageVG(0)
        stageVG(1)
        stageA(0)
        stage1(2)
        stageQK(0)
        stageT(0)
        stageVG(2)
        stageQK(1)
        stage1(3)
        stageT(1)
        stageVG(3)
        stageA(1)
        stageQK(2)
        stageT(2)
        stageQK(3)
        stageT(3)
    nc.finalize()
    return nc


_NC_CACHE = []


def _get_nc():
    if not _NC_CACHE:
        nc = bacc.Bacc("TRN2", target_bir_lowering=False, debug=False)
        _emit(nc)
        _NC_CACHE.append(nc)
    return _NC_CACHE[0]


def _bf16_split(a):
    import ml_dtypes
    hi = a.astype(ml_dtypes.bfloat16)
    lo = (a - hi.astype(np.float32)).astype(ml_dtypes.bfloat16)
    return np.ascontiguousarray(hi), np.ascontiguousarray(lo)


def _to_bf16(a):
    import ml_dtypes
    return np.ascontiguousarray(a.astype(ml_dtypes.bfloat16))


def _shard_inputs(x, W_qkv, W_a, W_g, W_out, b_out):
    x2 = np.ascontiguousarray(np.asarray(x, np.float32).reshape(N, D))
    W_qkv = np.asarray(W_qkv, np.float32)
    W_a = np.asarray(W_a, np.float32)
    W_g = np.asarray(W_g, np.float32)
    W_out = np.asarray(W_out, np.float32)

    xhi, xlo = _bf16_split(x2)
    idf = np.eye(128, dtype=np.float32)
    idb = _to_bf16(np.eye(128, dtype=np.float32))

    # W_a column permutation: within a core's 128 cols, source col
    # h*64 + 2d + c  ->  dest col c*64 + h*32 + d
    perm = np.empty(128, np.int64)
    for c in range(2):
        for h in range(2):
            for d in range(DC):
                perm[c * 64 + h * 32 + d] = h * 64 + 2 * d + c

    in_maps = []
    for r in range(NCORES):
        cs = r * 128
        wq = _to_bf16(W_qkv[:, cs:cs + 128] * np.float32(DH ** -0.5))
        wk = _to_bf16(W_qkv[:, D + cs:D + cs + 128])
        wv = _to_bf16(W_qkv[:, 2 * D + cs:2 * D + cs + 128])
        wah, wal = _bf16_split(
            np.ascontiguousarray(W_a[:, cs:cs + 128][:, perm]))
        wg = _to_bf16(W_g[:, cs:cs + 128])
        wo = _to_bf16(W_out[cs:cs + 128, :])
        in_maps.append({
            "XHI": xhi, "XLO": xlo, "WQ": wq, "WK": wk, "WAH": wah,
            "WAL": wal, "WV": wv, "WG": wg, "WO": wo,
            "IDF": idf, "IDB": idb,
        })
    return in_maps


def _unshard(results, b_out):
    outT = np.zeros((D, N), np.float32)
    for r in results:
        outT += r["OUT"]
    outT += np.asarray(b_out, np.float32).reshape(D, 1)
    return np.ascontiguousarray(outT.T).reshape(1, N, D)


def run(trace=False, **inputs):
    nc = _get_nc()
    in_maps = _shard_inputs(**inputs)
    res = run_bass_kernel_spmd(nc, in_maps, core_ids=list(range(NCORES)),
                               trace=trace)
    return _unshard(res.results, inputs["b_out"]), res


def kernel(**inputs) -> np.ndarray:
    out, _ = run(trace=False, **inputs)
    return out
